# revision 1
# baseline (speedup 1.0000x reference)
"""Trainium2 Bass kernel for nn_KeplerDiffEq.

Computes, per orbit (4 orbits on 4 SBUF partitions):
  E = Kepler solve (Newton, seed E0 = M + e*sinM, 4 iterations)
  dr/ddr via the orbital-plane -> inertial rotation, out = [dr | ddr]  [4,6]

Design notes:
  - All trig via the ACT Sin table only (one table set, one ~2.7us load).
    cos(t) is obtained as -sin(t - pi/2); every Sin argument is kept inside
    [-pi, pi] where the spline is accurate (verified on HW: err ~6e-8 inside,
    blows up outside).
  - No ACT Sqrt (different table set + 65536-ULP budget): sqrt/rsqrt via the
    magic-constant seed + 3 Newton-Raphson steps on the vector engine.
  - ||r|| uses orthonormality of the rotation columns: ||r||^2 = x^2 + y^2.
  - The 2000-step damped reference loop stalls in f32 ~5e-6 from the true
    root; a converged Newton solution matches it to ~1e-5 relative.

Sharding: problem is tiny ("too small to shard") -> replicated SPMD on all
8 cores; core 0's output is returned.
"""
import sys

if "/opt/trn_rl_repo" not in sys.path:
    sys.path.insert(0, "/opt/trn_rl_repo")

import numpy as np

N_ORBITS = 4
N_IN = 28
N_OUT = 6
N_NEWTON = 4
HALF_PI = float(np.float32(np.pi / 2))
MU = 3.0
MAGIC = 0x5F3759DF

_cache = {}


def _build():
    import concourse.tile as tile
    from concourse import bacc, mybir

    AF = mybir.ActivationFunctionType
    ALU = mybir.AluOpType
    F32 = mybir.dt.float32
    I32 = mybir.dt.int32
    P = N_ORBITS

    nc = bacc.Bacc("TRN2", target_bir_lowering=False, debug=False)
    IN = nc.dram_tensor("inp", [P, N_IN], F32, kind="ExternalInput")
    OUT = nc.dram_tensor("out", [P, N_OUT], F32, kind="ExternalOutput")

    with tile.TileContext(nc) as tc:
        with tc.tile_pool(name="p", bufs=1) as pool:
            tin = pool.tile([P, N_IN], F32, tag="tin")
            nc.sync.dma_start(tin[:], IN.ap())

            e_ap = tin[:, 11:12]
            a_ap = tin[:, 10:11]
            mm_ap = tin[:, 12:13]
            m_ap = tin[:, 0:1]
            xy_ap = tin[:, 13:15]

            # ANG = base angles + {0,-pi/2} offsets; T = sin(ANG)
            # T cols: [sinM, s_w, n_w, n_W, s_W, s_i, s_W, n_W, n_i, spare]
            # (s_* = sin, n_* = -cos via sin(t - pi/2))
            ang = pool.tile([P, 10], F32, tag="ang")
            nc.vector.tensor_tensor(out=ang[:], in0=tin[:, 0:10],
                                    in1=tin[:, 16:26], op=ALU.add)
            T = pool.tile([P, 10], F32, tag="T")
            nc.scalar.activation(T[:], ang[:], AF.Sin)

            # ---- rotation-matrix build (off the Newton critical path) ----
            # A cols 0-5 = [n_w, -n_w, s_w, s_w, -s_w, -n_w]; cols 6-9 =
            # A2m = [s_w, s_w, -n_w, -n_w]
            A = pool.tile([P, 10], F32, tag="A")
            nc.vector.tensor_copy(A[:, 0:1], T[:, 2:3])
            nc.vector.tensor_scalar(out=A[:, 1:6:4],
                                    in0=T[:, 2:3].broadcast_to([P, 2]),
                                    scalar1=-1.0, scalar2=None, op0=ALU.mult)
            nc.vector.tensor_copy(A[:, 2:4], T[:, 1:2].broadcast_to([P, 2]))
            nc.vector.tensor_scalar(out=A[:, 4:5], in0=T[:, 1:2],
                                    scalar1=-1.0, scalar2=None, op0=ALU.mult)
            nc.vector.tensor_copy(A[:, 6:8], T[:, 1:2].broadcast_to([P, 2]))
            nc.vector.tensor_scalar(out=A[:, 8:10],
                                    in0=T[:, 2:3].broadcast_to([P, 2]),
                                    scalar1=-1.0, scalar2=None, op0=ALU.mult)

            # C = [c11,c21,c31,c12,c22,c32]: first factors A6 * [n_W,s_W,s_i]x2
            C = pool.tile([P, 6], F32, tag="C")
            B_b = T[:, 3:6].unsqueeze(1).broadcast_to([P, 2, 3])
            nc.vector.tensor_tensor(
                out=C[:].rearrange("p (h j) -> p h j", h=2),
                in0=A[:, 0:6].rearrange("p (h j) -> p h j", h=2),
                in1=B_b, op=ALU.mult)
            # second terms on cols (0,1,3,4):
            p2 = pool.tile([P, 2], F32, tag="p2")
            nc.vector.tensor_scalar(out=p2[:], in0=T[:, 6:8],
                                    scalar1=T[:, 8:9], scalar2=None,
                                    op0=ALU.mult)
            Gm = pool.tile([P, 4], F32, tag="Gm")
            nc.vector.tensor_tensor(
                out=Gm[:].rearrange("p (h j) -> p h j", h=2),
                in0=A[:, 6:10].rearrange("p (h j) -> p h j", h=2),
                in1=p2[:].unsqueeze(1).broadcast_to([P, 2, 2]), op=ALU.mult)
            C2 = pool.tile([P, 6], F32, tag="C2")
            main4_in = C[:].rearrange("p (h j) -> p h j", h=2)[:, :, 0:2]
            main4_out = C2[:].rearrange("p (h j) -> p h j", h=2)[:, :, 0:2]
            nc.vector.tensor_tensor(out=main4_out, in0=main4_in,
                                    in1=Gm[:].rearrange("p (h j) -> p h j", h=2),
                                    op=ALU.add)
            nc.vector.tensor_copy(C2[:, 2:6:3], C[:, 2:6:3])
            C = C2

            # ---- V = [MU*a, MU*a*(1-e^2), x^2+y^2]; Y = rsqrt(V) ----
            V = pool.tile([P, 3], F32, tag="V")
            nc.vector.tensor_scalar(out=V[:, 0:1], in0=a_ap, scalar1=MU,
                                    scalar2=None, op0=ALU.mult)
            e2t = pool.tile([P, 1], F32, tag="e2t")
            nc.vector.tensor_scalar(out=e2t[:], in0=tin[:, 11:12],
                                    scalar1=e_ap, scalar2=None, op0=ALU.mult)
            ome2 = pool.tile([P, 1], F32, tag="ome2")
            nc.vector.tensor_tensor(out=ome2[:], in0=tin[:, 15:16],
                                    in1=e2t[:], op=ALU.subtract)
            nc.vector.tensor_tensor(out=V[:, 1:2], in0=V[:, 0:1],
                                    in1=ome2[:], op=ALU.mult)
            sqxy = pool.tile([P, 2], F32, tag="sqxy")
            nc.vector.tensor_tensor(out=sqxy[:], in0=xy_ap, in1=xy_ap,
                                    op=ALU.mult)
            nc.vector.tensor_tensor(out=V[:, 2:3], in0=sqxy[:, 0:1],
                                    in1=sqxy[:, 1:2], op=ALU.add)

            Y = pool.tile([P, 3], F32, tag="Y")
            sh = pool.tile([P, 3], I32, tag="sh")
            nc.vector.tensor_scalar(out=sh[:], in0=V[:].bitcast(I32),
                                    scalar1=1, scalar2=None,
                                    op0=ALU.logical_shift_right)
            nc.vector.tensor_scalar(out=Y[:].bitcast(I32), in0=sh[:],
                                    scalar1=MAGIC, scalar2=-1,
                                    op0=ALU.subtract, op1=ALU.mult)
            for it in range(3):
                t_a = pool.tile([P, 3], F32, tag=f"nra{it}")
                nc.vector.tensor_tensor(out=t_a[:], in0=Y[:], in1=Y[:],
                                        op=ALU.mult)
                nc.vector.tensor_tensor(out=t_a[:], in0=t_a[:], in1=V[:],
                                        op=ALU.mult)
                nc.vector.tensor_scalar(out=t_a[:], in0=t_a[:],
                                        scalar1=-0.5, scalar2=1.5,
                                        op0=ALU.mult, op1=ALU.add)
                Y2 = pool.tile([P, 3], F32, tag=f"nry{it}")
                nc.vector.tensor_tensor(out=Y2[:], in0=Y[:], in1=t_a[:],
                                        op=ALU.mult)
                Y = Y2
            SQ = pool.tile([P, 2], F32, tag="SQ")
            nc.vector.tensor_tensor(out=SQ[:], in0=V[:, 0:2], in1=Y[:, 0:2],
                                    op=ALU.mult)
            t1k = pool.tile([P, 1], F32, tag="t1k")
            nc.vector.tensor_scalar(out=t1k[:], in0=mm_ap, scalar1=mm_ap,
                                    scalar2=a_ap, op0=ALU.mult, op1=ALU.mult)
            t2k = pool.tile([P, 1], F32, tag="t2k")
            nc.vector.tensor_scalar(out=t2k[:], in0=t1k[:], scalar1=a_ap,
                                    scalar2=a_ap, op0=ALU.mult, op1=ALU.mult)

            # ---- Newton-Kepler: EE = [F, F - pi/2], F = E - M ----
            P0 = pool.tile([P, 1], F32, tag="P0")
            nc.vector.tensor_scalar(out=P0[:], in0=T[:, 0:1], scalar1=e_ap,
                                    scalar2=None, op0=ALU.mult)
            EE = pool.tile([P, 2], F32, tag="EE0")
            nc.vector.tensor_tensor(out=EE[:], in0=P0[:].broadcast_to([P, 2]),
                                    in1=tin[:, 16:19:2], op=ALU.add)

            S = None
            for it in range(N_NEWTON):
                S = pool.tile([P, 2], F32, tag=f"S{it}")
                nc.scalar.activation(S[:], EE[:], AF.Sin, bias=m_ap)
                nnum = pool.tile([P, 1], F32, tag=f"nn{it}")
                nc.vector.tensor_scalar(out=nnum[:], in0=S[:, 0:1],
                                        scalar1=e_ap, scalar2=EE[:, 0:1],
                                        op0=ALU.mult, op1=ALU.subtract)
                den = pool.tile([P, 1], F32, tag=f"dn{it}")
                nc.vector.tensor_scalar(out=den[:], in0=S[:, 1:2],
                                        scalar1=e_ap, scalar2=1.0,
                                        op0=ALU.mult, op1=ALU.add)
                rec = pool.tile([P, 1], F32, tag=f"rc{it}")
                nc.vector.reciprocal(rec[:], den[:])
                dF = pool.tile([P, 1], F32, tag=f"dF{it}")
                nc.vector.tensor_tensor(out=dF[:], in0=nnum[:], in1=rec[:],
                                        op=ALU.mult)
                EE2 = pool.tile([P, 2], F32, tag=f"EE{it + 1}")
                nc.vector.tensor_tensor(out=EE2[:], in0=EE[:],
                                        in1=dF[:].broadcast_to([P, 2]),
                                        op=ALU.add)
                EE = EE2

            # final trig at converged E
            S5 = pool.tile([P, 2], F32, tag="S5")
            nc.scalar.activation(S5[:], EE[:], AF.Sin, bias=m_ap)
            den5 = pool.tile([P, 1], F32, tag="den5")
            nc.vector.tensor_scalar(out=den5[:], in0=S5[:, 1:2],
                                    scalar1=e_ap, scalar2=1.0,
                                    op0=ALU.mult, op1=ALU.add)

            # ---- tail ----
            rcen = pool.tile([P, 1], F32, tag="rcen")
            nc.vector.tensor_scalar(out=rcen[:], in0=den5[:], scalar1=a_ap,
                                    scalar2=None, op0=ALU.mult)
            rcinv = pool.tile([P, 1], F32, tag="rcinv")
            nc.vector.reciprocal(rcinv[:], rcen[:])

            sc2 = pool.tile([P, 2], F32, tag="sc2")
            nc.vector.tensor_scalar(out=sc2[:], in0=SQ[:], scalar1=rcinv[:],
                                    scalar2=None, op0=ALU.mult)
            ds2 = pool.tile([P, 2], F32, tag="ds2")
            nc.vector.tensor_tensor(out=ds2[:], in0=sc2[:], in1=S5[:],
                                    op=ALU.mult)
            PQ = pool.tile([P, 4], F32, tag="PQ")
            nc.vector.tensor_scalar(out=PQ[:, 0:4:2], in0=ds2[:],
                                    scalar1=-1.0, scalar2=None, op0=ALU.mult)
            t3k = pool.tile([P, 1], F32, tag="t3k")
            nc.vector.tensor_scalar(out=t3k[:], in0=t2k[:], scalar1=rcinv[:],
                                    scalar2=rcinv[:], op0=ALU.mult,
                                    op1=ALU.mult)
            kk = pool.tile([P, 1], F32, tag="kk")
            nc.vector.tensor_scalar(out=kk[:], in0=t3k[:], scalar1=Y[:, 2:3],
                                    scalar2=-1.0, op0=ALU.mult, op1=ALU.mult)
            nc.vector.tensor_tensor(out=PQ[:, 1:4:2],
                                    in0=kk[:].broadcast_to([P, 2]),
                                    in1=xy_ap, op=ALU.mult)

            O1 = pool.tile([P, 6], F32, tag="O1")
            nc.vector.tensor_tensor(
                out=O1[:].rearrange("p (h j) -> p h j", h=2),
                in0=C[:, 0:3].unsqueeze(1).broadcast_to([P, 2, 3]),
                in1=PQ[:, 0:2].unsqueeze(2).broadcast_to([P, 2, 3]),
                op=ALU.mult)
            O2 = pool.tile([P, 6], F32, tag="O2")
            nc.vector.tensor_tensor(
                out=O2[:].rearrange("p (h j) -> p h j", h=2),
                in0=C[:, 3:6].unsqueeze(1).broadcast_to([P, 2, 3]),
                in1=PQ[:, 2:4].unsqueeze(2).broadcast_to([P, 2, 3]),
                op=ALU.mult)
            Ot = pool.tile([P, 6], F32, tag="Ot")
            nc.vector.tensor_tensor(out=Ot[:], in0=O1[:], in1=O2[:],
                                    op=ALU.add)
            nc.sync.dma_start(OUT.ap(), Ot[:])

    nc.compile()
    return nc


def _pack(a, e, i, omega, Omega, mean_motion, mean_anomaly, x):
    P = N_ORBITS
    IN = np.zeros((P, N_IN), np.float32)
    M = np.full((P,), np.float32(mean_anomaly), np.float32)
    w = np.asarray(omega, np.float32).reshape(P)
    W = np.asarray(Omega, np.float32).reshape(P)
    ii = np.asarray(i, np.float32).reshape(P)
    IN[:, 0] = M
    IN[:, 1] = w
    IN[:, 2] = w
    IN[:, 3] = W
    IN[:, 4] = W
    IN[:, 5] = ii
    IN[:, 6] = W
    IN[:, 7] = W
    IN[:, 8] = ii
    IN[:, 9] = M
    IN[:, 10] = np.asarray(a, np.float32).reshape(P)
    IN[:, 11] = np.asarray(e, np.float32).reshape(P)
    IN[:, 12] = np.asarray(mean_motion, np.float32).reshape(P)
    IN[:, 13] = np.asarray(x, np.float32)[:, 0]
    IN[:, 14] = np.asarray(x, np.float32)[:, 1]
    IN[:, 15] = 1.0
    IN[:, 16:26] = np.array([0, 0, -HALF_PI, -HALF_PI, 0,
                             0, 0, -HALF_PI, -HALF_PI, 0], np.float32)
    return IN


def kernel(a, e, i, omega, Omega, mean_motion, mean_anomaly, x, _trace=False):
    from concourse.bass_utils import run_bass_kernel_spmd

    if "nc" not in _cache:
        _cache["nc"] = _build()
    nc = _cache["nc"]

    IN = _pack(a, e, i, omega, Omega, mean_motion, mean_anomaly, x)
    n_cores = 1 if _trace else 8
    res = run_bass_kernel_spmd(nc, [{"inp": IN}] * n_cores,
                               core_ids=list(range(n_cores)), trace=_trace)
    out = res.results[0]["out"].astype(np.float32)
    if _trace:
        _cache["last_result"] = res
    return out

